# revision 15
# baseline (speedup 1.0000x reference)
"""Multi-hot embedding bag kernel for Trainium2 (8 NeuronCores, batch-sharded).

Computes, for 5 feature groups g with multi-hot int32 matrices A_g [B, V_g]
and weights W_g [V_g, 64]:
    out = concat_g(norm_g(A_g @ W_g))  with the original module's quirks:
    - "decades" is normalized by its own row-sum AND by the movie row-sum
    - "movies" is never normalized
    - remaining groups are normalized by their own row-sum (rows with sum 0
      are left unnormalized)

Strategy (per core, 256 batch rows):
  - The multi-hot values are exactly {0, 1}, so the host pre-packs each A_g
    TRANSPOSED into fp8e4 (0.0 / 1.0 are exact in e4m3) with a
    partition-major chunk layout [128, C, 256]: partition p / chunk c /
    batch col b holds A_g[b, c*128 + p].
  - Weights are host-packed as [W_g | 1] chunks [128, C, 65]; the ones
    column makes the matmul emit row-sums for free. The movie weights set
    the output scale (movies are never normalized) and stay fp16; every
    other group's weights are fp8e4 scaled by 32 (the 1/32 folds into the
    normalization).
  - fp8-weight groups run PAIRS of chunks per matmul in DoubleRow perf
    mode (lhsT [128,2,65] fp8, rhs [128,2,256] fp8 -> out [65,256]),
    roughly halving tensor-engine time for 2/3 of the chunks so the PE
    stays off the critical path.
  - DMA is strictly consumption-ordered: for each idx slab, its weight
    piece(s) are issued immediately before it ON THE SAME QUEUE, and slabs
    alternate between the two HW DGE queues (sync/SP and scalar/Act).
    Weight arrival can then never lag consumption by more than the slab
    lookahead, unlike a dedicated weight queue which gets an unneeded
    bandwidth share early and starves the PE mid-stream.
  - Group end: copy PSUM accumulator to SBUF, transpose back on the PE
    (fp32 identity), normalize with per-row reciprocals, and DMA that
    group's [128, 64] output block immediately (per batch-half), so the
    end-of-kernel tail is only the LAST group's finalize.
"""

import math

import numpy as np

import concourse.bass as bass
import concourse.tile as tile
from concourse import bacc, mybir
from concourse.bass_utils import run_bass_kernel_spmd
from concourse.masks import make_identity

B = 2048
LF = 64
FE = LF + 1  # weights + ones column
N_CORES = 8
BPC = B // N_CORES  # 256 batch rows per core
P = 128
W8_SCALE = 32.0  # fp8 weight groups are stored as 32*W to dodge denormals

# (key, idx input name, weight input name, vocab size, output column offset,
#  fp8 weights?)
GROUPS = [
    ("dec", "decade_idxs", "W_dec", 12, 0, True),
    ("cat", "category_idxs", "W_cat", 32, 128, True),
    ("com", "company_idxs", "W_com", 20000, 256, True),
    ("per", "person_idxs", "W_per", 100000, 192, True),
    ("mov", "movie_idxs", "W_mov", 60000, 64, False),
]
OUT_COLS = 5 * LF
NCH = {g[0]: math.ceil(g[3] / P) for g in GROUPS}
CTOT = sum(NCH.values())
C8 = sum(NCH[g[0]] for g in GROUPS if g[5])
C16 = CTOT - C8  # movie chunks

_FP8 = mybir.dt.float8e4
_FP16 = mybir.dt.float16
_FP32 = mybir.dt.float32

# fp8 weight chunks are stored on a stride of 80 elements (65 used): the
# DoubleRow LDWEIGHTS ISA check requires the k-tile pair step to be a
# multiple of 16 elements.
W8S = 80

MAX_SLAB = 72  # chunks per slab
TAIL = 96  # movie-free chunks at the end of the stream
GI = {g[0]: i for i, g in enumerate(GROUPS)}


def _units():
    """Unit stream: DR pairs / singles for fp8 groups in order
    dec,cat,com,per; movie singles interleaved evenly so movies finish
    TAIL chunks before the stream ends.

    Returns a list of units (is8, gi, si, nch, is_start, is_stop) where si
    is the index of the unit's first chunk in its dtype-stream (fp8 or
    fp16 stream order) and nch in (1, 2)."""
    u8 = []
    s = 0
    for key in ("dec", "cat", "com", "per"):
        gi, n = GI[key], NCH[key]
        j = 0
        while j < n:
            take = 2 if j + 1 < n else 1
            u8.append((True, gi, s + j, take, j == 0, j + take == n))
            j += take
        s += n
    u16 = [(False, GI["mov"], j, 1, j == 0, j == C16 - 1) for j in range(C16)]

    # merge movie chunks in blocks of MBLK and fp8 units in blocks of
    # ~2*MBLK chunks: alternating stationary types (DoubleRow vs plain)
    # breaks LDWEIGHTS pipelining, so keep runs homogeneous. Movies still
    # finish TAIL chunks before the stream ends.
    MBLK = 16
    L = CTOT - TAIL
    order, i8, i16, pos = [], 0, 0, 0
    while i8 < len(u8) or i16 < len(u16):
        if i16 < len(u16) and (
                i8 >= len(u8) or i16 + 1 <= (pos + 1) * C16 // L):
            take = min(MBLK, len(u16) - i16)
            for _ in range(take):
                order.append(u16[i16])
                i16 += 1
                pos += 1
        else:
            emitted = 0
            while i8 < len(u8) and emitted < 2 * MBLK:
                order.append(u8[i8])
                pos += u8[i8][3]
                emitted += u8[i8][3]
                i8 += 1
    return order


def _slab_plan(units):
    """Pack units into slabs; returns list of lists of units. Leading
    slabs are small so the first matmul starts fast."""
    caps = [8, 16, 32] + [MAX_SLAB] * 10000
    slabs, cur, cnt, k = [], [], 0, 0
    for u in units:
        if cnt + u[3] > caps[k]:
            slabs.append(cur)
            cur, cnt, k = [], 0, k + 1
        cur.append(u)
        cnt += u[3]
    if cur:
        slabs.append(cur)
    return slabs


UNITS = _units()
SLABS = _slab_plan(UNITS)


def _build() -> bass.Bass:
    nc = bacc.Bacc(None, target_bir_lowering=False)

    a_dram = nc.dram_tensor("a_all", [P, CTOT * BPC], _FP8, kind="ExternalInput")
    w8_dram = nc.dram_tensor("w8", [P, C8 * W8S], _FP8, kind="ExternalInput")
    w16_dram = nc.dram_tensor("w16", [P, C16 * FE], _FP16, kind="ExternalInput")
    out = nc.dram_tensor("out", [BPC, OUT_COLS], _FP32, kind="ExternalOutput")

    with tile.TileContext(nc) as tc:
        with (
            tc.tile_pool(name="singles", bufs=1) as singles,
            tc.tile_pool(name="apool", bufs=3) as apool,
            tc.tile_pool(name="npool", bufs=4) as npool,
            tc.tile_pool(name="accp", bufs=2, space="PSUM") as accp,
            tc.tile_pool(name="decp", bufs=1, space="PSUM") as decp,
            tc.tile_pool(name="movp", bufs=1, space="PSUM") as movp,
            tc.tile_pool(name="backp", bufs=2, space="PSUM") as backp,
        ):
            # Per-slab DMAs in consumption order, alternating queues.
            # Weight pieces for slab k go right before slab k's idx DMA on
            # the same queue; every weight tile stays resident.
            qs = [nc.sync, nc.scalar]
            w8p, w16p, a_sb = [], [], []
            c0 = s8 = s16 = 0
            for k, slab in enumerate(SLABS):
                q = qs[k % 2]
                n8 = sum(u[3] for u in slab if u[0])
                n16 = sum(u[3] for u in slab if not u[0])
                ch = n8 + n16
                t8 = t16 = None
                if n8:
                    t8 = singles.tile([P, n8, W8S], _FP8, name=f"w8p{k}")
                    q.dma_start(
                        t8,
                        w8_dram[:, s8 * W8S:(s8 + n8) * W8S].rearrange(
                            "p (c f) -> p c f", f=W8S))
                if n16:
                    t16 = singles.tile([P, n16, FE], _FP16, name=f"w16p{k}")
                    q.dma_start(
                        t16,
                        w16_dram[:, s16 * FE:(s16 + n16) * FE].rearrange(
                            "p (c f) -> p c f", f=FE))
                w8p.append((t8, s8))
                w16p.append((t16, s16))
                at = apool.tile([P, MAX_SLAB, BPC], _FP8, tag="a")
                q.dma_start(
                    at[:, :ch, :],
                    a_dram[:, c0 * BPC:(c0 + ch) * BPC].rearrange(
                        "p (c b) -> p c b", b=BPC))
                a_sb.append(at)
                c0 += ch
                s8 += n8
                s16 += n16

            ident32 = singles.tile([P, P], _FP32)
            make_identity(nc, ident32)

            rmov = [singles.tile([P, 1], _FP32, name=f"rmov{i}")
                    for i in range(2)]

            def finalize(gi, accT):
                key, _, _, _, col, is8 = GROUPS[gi]
                accT_sb = npool.tile([FE, 2 * P], _FP32, tag="accsb")
                nc.vector.tensor_copy(accT_sb, accT)
                for bt in range(2):
                    out2 = backp.tile([P, FE], _FP32, tag="out2")
                    nc.tensor.matmul(
                        out2,
                        lhsT=accT_sb[:, bass.ts(bt, P)],
                        rhs=ident32[:FE, :FE],
                        start=True, stop=True,
                    )
                    s = npool.tile([P, 1], _FP32, tag="s")
                    nc.vector.tensor_scalar_max(s, out2[:, LF:FE], 1.0)
                    if is8:
                        # emb rows carry 32*W; divide by 32*max(sum, 1)
                        nc.vector.tensor_scalar_mul(s, s, W8_SCALE)
                    nc.vector.reciprocal(s, s)
                    ob = npool.tile([P, LF], _FP32, tag="ob")
                    if key == "mov":
                        # movies stay unnormalized; stash 1/max(sum,1) for
                        # the decades double-normalization
                        nc.vector.tensor_copy(rmov[bt], s)
                        nc.scalar.copy(ob, out2[:, :LF])
                    else:
                        if key == "dec":
                            nc.vector.tensor_mul(s, s, rmov[bt])
                        nc.vector.tensor_scalar_mul(ob, out2[:, :LF], s)
                    nc.sync.dma_start(
                        out[bt * P:(bt + 1) * P, col:col + LF], ob)

            accs = {}  # group index -> live PSUM tile
            for k, slab in enumerate(SLABS):
                at = a_sb[k]
                t8, sl8 = w8p[k]
                t16, sl16 = w16p[k]
                j = j8 = j16 = 0
                for is8, gi, si, nch, is_start, is_stop in slab:
                    key = GROUPS[gi][0]
                    if is_start:
                        pool = {"dec": decp, "mov": movp}.get(key, accp)
                        accs[gi] = pool.tile([FE, 2 * P], _FP32, tag="acc",
                                             name=f"acc_{key}")
                    if is8:
                        w_sb = t8[:, j8:j8 + nch, :FE]
                        j8 += nch
                    else:
                        w_sb = t16[:, j16:j16 + nch, :]
                        j16 += nch
                    if nch == 2:
                        nc.tensor.matmul(
                            accs[gi],
                            lhsT=w_sb,
                            rhs=at[:, j:j + 2, :],
                            start=is_start,
                            stop=is_stop,
                            perf_mode=mybir.MatmulPerfMode.DoubleRow,
                        )
                    else:
                        nc.tensor.matmul(
                            accs[gi],
                            lhsT=w_sb[:, 0, :],
                            rhs=at[:, j, :],
                            start=is_start,
                            stop=is_stop,
                        )
                    j += nch
                    if is_stop and key != "dec":
                        finalize(gi, accs[gi])
                        if key == "mov":
                            # rmov now exists; decades finalizes here and
                            # overlaps the remaining fp8 chunks
                            finalize(GI["dec"], accs[GI["dec"]])

    nc.finalize()
    return nc


_NC_CACHE: bass.Bass | None = None


def _get_nc() -> bass.Bass:
    global _NC_CACHE
    if _NC_CACHE is None:
        _NC_CACHE = _build()
    return _NC_CACHE


def _pack_weights(w: np.ndarray, fp8: bool) -> np.ndarray:
    """[V, 64] fp32 -> [128, C*stride] (fp16 stride 65, or fp8 scaled by 32
    on stride 80) with ones column and zero padding, laid out so chunk c /
    partition p / feature f = row c*128+p of [W | 1]."""
    import ml_dtypes

    v = w.shape[0]
    c = math.ceil(v / P)
    scale = W8_SCALE if fp8 else 1.0
    stride = W8S if fp8 else FE
    we = np.concatenate([w.astype(np.float32) * scale,
                        np.ones((v, 1), np.float32)], axis=1)
    if c * P > v:
        we = np.concatenate([we, np.zeros((c * P - v, FE), np.float32)], axis=0)
    if stride > FE:
        we = np.concatenate(
            [we, np.zeros((c * P, stride - FE), np.float32)], axis=1)
    we = we.astype(ml_dtypes.float8_e4m3 if fp8 else np.float16)
    return np.ascontiguousarray(
        we.reshape(c, P, stride).transpose(1, 0, 2).reshape(P, c * stride))


def _pack_idx_group(x: np.ndarray) -> np.ndarray:
    """[B, V] int32 {0,1} -> [8, 128, C, 256] uint8 fp8e4 bit patterns,
    element (core, p, c, b) = 0x38 * x[core*256 + b, c*128 + p]."""
    v = x.shape[1]
    c = math.ceil(v / P)
    xb = (x != 0).astype(np.uint8) * np.uint8(0x38)
    if c * P > v:
        xb = np.concatenate(
            [xb, np.zeros((B, c * P - v), np.uint8)], axis=1)
    # [B, C*128] -> [8 cores, 256 b, C, 128 p] -> [8, 128, C, 256]
    return np.ascontiguousarray(
        xb.reshape(N_CORES, BPC, c, P).transpose(0, 3, 2, 1))


def kernel(**inputs: np.ndarray) -> np.ndarray:
    import os

    import ml_dtypes

    nc = _get_nc()

    # weight tensors in dtype-stream order (= group order dec,cat,com,per
    # for fp8; movie for fp16), which matches unit emission order
    w8 = np.concatenate(
        [_pack_weights(np.asarray(inputs[wn]), True)
         for _, _, wn, _, _, f8 in GROUPS if f8], axis=1)
    w16 = np.concatenate(
        [_pack_weights(np.asarray(inputs[wn]), False)
         for _, _, wn, _, _, f8 in GROUPS if not f8], axis=1)
    a8 = [_pack_idx_group(np.asarray(inputs[an]))
          for _, an, _, _, _, f8 in GROUPS if f8]
    a16 = [_pack_idx_group(np.asarray(inputs[an]))
           for _, an, _, _, _, f8 in GROUPS if not f8]

    # global chunk permutation: stream chunks in merged unit order
    perm = np.empty(CTOT, np.int64)
    pos = 0
    for is8, gi, si, nch, _, _ in UNITS:
        for t in range(nch):
            perm[pos] = si + t if is8 else C8 + si + t
            pos += 1
    assert pos == CTOT

    in_maps = []
    for core in range(N_CORES):
        a_core = np.concatenate(
            [p[core] for p in a8] + [p[core] for p in a16], axis=1)
        a_core = np.ascontiguousarray(a_core[:, perm, :])
        in_maps.append({
            "a_all": a_core.reshape(P, CTOT * BPC).view(ml_dtypes.float8_e4m3),
            "w8": w8,
            "w16": w16,
        })

    trace = bool(int(os.environ.get("EMB_TRACE", "0")))
    res = run_bass_kernel_spmd(nc, in_maps, core_ids=list(range(N_CORES)),
                               trace=trace)
    if trace and res.exec_time_ns is not None:
        print(f"HW exec time: {res.exec_time_ns} ns")
        if res.instructions_and_trace is not None:
            print(f"trace: {res.instructions_and_trace[1]}")

    return np.concatenate([r["out"] for r in res.results], axis=0)
